# revision 35
# baseline (speedup 1.0000x reference)
import numpy as np
import sys
sys.path.insert(0, '/opt/trn_rl_repo')
import jax
try:
    jax.config.update("jax_compilation_cache_dir", "/tmp/jaxcache")
    jax.config.update("jax_persistent_cache_min_compile_time_secs", 0.0)
    jax.config.update("jax_persistent_cache_min_entry_size_bytes", 0)
except Exception:
    pass
import concourse.bacc as bacc
import concourse.mybir as mybir
from concourse.tile import TileContext
from concourse.bass_utils import run_bass_kernel_spmd
import concourse.tile_utils as tile_utils
tile_utils.max_sbuf_usage = 207 * 1024

import ml_dtypes
BF = ml_dtypes.bfloat16

TH1 = 2.3599835635698114
TH2 = 7.985043705972782
TH3 = 3.849629060468402
BETA = 0.44154740154430405
EPS = 1e-5
NSTEP = 10
NCORES = 8
B = 512            # batch per core
F32 = mybir.dt.float32
BF16 = mybir.dt.bfloat16

# Even conv-rows first inside each 8-row block so 2x2 pool-y is a single
# quadrant-aligned max(p[0:64], p[64:128]).
PERM8 = (0, 2, 4, 6, 1, 3, 5, 7)

_cache = {}
LAST_RES = None
LAST_NS = -1


def _build_program():
    nc = bacc.Bacc("TRN2", target_bir_lowering=False, debug=False, num_devices=NCORES)

    x26_d = nc.dram_tensor("x26", [26, 26 * B], F32, kind="ExternalInput")
    bw1c_d = nc.dram_tensor("bw1c", [3, 3, 16], F32, kind="ExternalInput")
    # aux32 cols: 0:3 bn1 (m,s,b), 3:6 bn2, 6 b_fc (rows 0:10)
    aux32_d = nc.dram_tensor("aux32", [128, 7], F32, kind="ExternalInput")
    # auxbf cols: 0:96 bw2c (3 x [48,32], rows 0:48), 96:146 wfc0123, 146:196 wfc4 (rows 0:32)
    auxbf_d = nc.dram_tensor("auxbf", [128, 196], BF16, kind="ExternalInput")
    # rows 0..99: mem3 per step; 100..109: spike bits t=0..7; 110..119: t=8,9
    out_d = nc.dram_tensor("out", [120, B], BF16, kind="ExternalOutput")

    GT, MUL, ADD, SUB, MAX = (mybir.AluOpType.is_gt, mybir.AluOpType.mult,
                              mybir.AluOpType.add, mybir.AluOpType.subtract,
                              mybir.AluOpType.max)
    F = 12 * B         # LIF1 free width: X' 0..11 (X'=12 never read by conv2)

    with TileContext(nc) as tc:
        with (
            tc.tile_pool(name="state", bufs=1) as st,
            tc.tile_pool(name="xt", bufs=1) as xpool,
            tc.tile_pool(name="tr", bufs=1) as tr,
            tc.tile_pool(name="tq", bufs=1) as tq,
            tc.tile_pool(name="outp", bufs=1) as op,
            tc.tile_pool(name="ps", bufs=2, space="PSUM") as pp,
        ):
            cur1a = st.tile([128, F], F32)      # Y' 0..7 x16ch
            cur1b = st.tile([64, F], F32)       # Y' 8..11
            mem1a = st.tile([128, F], F32)
            mem1b = st.tile([64, F], F32)
            spk1a = st.tile([128, F], BF16)
            spk1b = st.tile([64, F], BF16)
            m2ab = st.tile([128, 5 * B], F32)   # Y2 0,1 | 2,3
            m2c = st.tile([32, 5 * B], F32)     # Y2 4
            mem3 = st.tile([10, B], F32)

            aux32 = st.tile([128, 7], F32)
            nc.sync.dma_start(aux32[:], aux32_d[:])

            # conv1 stationaries built on device from the 144-value bw1c
            wc1t = st.tile([26, 9 * 128], F32)
            nc.vector.memset(wc1t[:], 0.0)
            for bi, y0 in enumerate((0, 8, 16)):
                for dx in range(3):
                    cb = (bi * 3 + dx) * 128
                    for p_, j in enumerate(PERM8):
                        nc.sync.dma_start(
                            wc1t[y0 + j:y0 + j + 3, cb + p_ * 16:cb + (p_ + 1) * 16],
                            bw1c_d[dx])

            # conv2 Toeplitz blocks built on device from auxbf cols 0:96
            w03, w47, w89 = [], [], []
            for dx in range(3):
                ws = slice(dx * 32, (dx + 1) * 32)
                t1 = st.tile([96, 128], BF16, tag=f"w03_{dx}")
                nc.vector.memset(t1[:], 0.0)
                for pos, yr in enumerate((0, 2, 1, 3)):
                    nc.sync.dma_start(t1[yr * 16:yr * 16 + 48, pos * 32:(pos + 1) * 32],
                                      auxbf_d[0:48, ws])
                w03.append(t1)
                # rows for Y 4..7 live at partitions 64..127, Y 8..9 at 0..31
                t2 = st.tile([128, 128], BF16, tag=f"w47_{dx}")
                nc.vector.memset(t2[:], 0.0)
                for pos, yr in enumerate((0, 2, 1, 3)):
                    r0 = yr * 16          # block row where this 48-row strip starts
                    cs = slice(pos * 32, (pos + 1) * 32)
                    lo = min(48, 64 - r0)  # rows landing in block rows <64 -> +64
                    if lo > 0:
                        nc.sync.dma_start(t2[64 + r0:64 + r0 + lo, cs], auxbf_d[0:lo, ws])
                    if lo < 48:
                        nc.sync.dma_start(t2[r0 + lo - 64:r0 + 48 - 64, cs], auxbf_d[lo:48, ws])
                w47.append(t2)
                t3 = st.tile([64, 64], BF16, tag=f"w89_{dx}")
                nc.vector.memset(t3[:], 0.0)
                for pos, yr in enumerate((0, 1)):
                    nc.sync.dma_start(t3[yr * 16:yr * 16 + 48, pos * 32:(pos + 1) * 32],
                                      auxbf_d[0:48, ws])
                w89.append(t3)
            wfc0123 = st.tile([128, 50], BF16)
            nc.sync.dma_start(wfc0123[:], auxbf_d[:, 96:146])
            wfc4 = st.tile([32, 50], BF16)
            nc.sync.dma_start(wfc4[:], auxbf_d[0:32, 146:196])
            acc07 = st.tile([10, B], F32)
            acc89 = st.tile([10, B], F32)
            nc.vector.memset(acc07[:], 0.0)
            nc.vector.memset(acc89[:], 0.0)

            nc.vector.memset(mem1a[:], 0.0)
            nc.vector.memset(mem1b[:], 0.0)
            nc.vector.memset(m2ab[:], 0.0)
            nc.vector.memset(m2c[:], 0.0)
            nc.vector.memset(mem3[:], 0.0)

            # ---- conv1 + 2x2 pool, on-device; three x-thirds to bound SBUF
            for h in range(3):
                xoff = 8 * h                    # x_in of tile column 0
                x26t = xpool.tile([26, 10 * B], F32, tag="x26")
                nc.sync.dma_start(x26t[:], x26_d[:, xoff * B:(xoff + 10) * B])
                for xp in range(4 * h, 4 * (h + 1)):
                    px0 = tr.tile([128, B], F32, tag="cpx0")
                    px1 = tr.tile([128, B], F32, tag="cpx1")
                    px2 = tr.tile([128, B], F32, tag="cpx2")
                    px = (px0, px1, px2)
                    for xo in range(2):
                        x = 2 * xp + xo
                        for bi in range(3):
                            p = pp.tile([128, B], F32, tag=("p03", "p47", "p89")[bi])
                            for dx in range(3):
                                ci = (x + dx - xoff) * B
                                nc.tensor.matmul(
                                    p[:],
                                    wc1t[:, (bi * 3 + dx) * 128:(bi * 3 + dx + 1) * 128],
                                    x26t[:, ci:ci + B],
                                    start=(dx == 0), stop=(dx == 2))
                            if xo == 0:
                                nc.scalar.copy(px[bi][:], p[:])
                            else:
                                nc.vector.tensor_tensor(px[bi][:], px[bi][:], p[:], op=MAX)
                    xs = slice(xp * B, (xp + 1) * B)
                    odc = tr.tile([64, B], F32, tag="odc")
                    for bi, dest in ((0, cur1a[0:64, xs]),
                                     (1, cur1a[64:128, xs]),
                                     (2, cur1b[0:64, xs])):
                        nc.vector.tensor_copy(odc[:], px[bi][64:128, :])
                        nc.vector.tensor_tensor(dest, px[bi][0:64, :], odc[:], op=MAX)

            # BN1 (pool-before-BN is exact: s1 >= 0)
            nc.vector.tensor_scalar(cur1a[:], cur1a[:], aux32[:, 0:1], aux32[:, 1:2], op0=SUB, op1=MUL)
            nc.vector.tensor_scalar(cur1a[:], cur1a[:], aux32[:, 2:3], None, op0=ADD)
            nc.vector.tensor_scalar(cur1b[:], cur1b[:], aux32[0:64, 0:1], aux32[0:64, 1:2], op0=SUB, op1=MUL)
            nc.vector.tensor_scalar(cur1b[:], cur1b[:], aux32[0:64, 2:3], None, op0=ADD)

            NCH = 4
            CW = F // NCH
            for t in range(NSTEP):
                # ---- LIF1
                for (mem, cur, spk, P) in ((mem1a, cur1a, spk1a, 128),
                                           (mem1b, cur1b, spk1b, 64)):
                    for hh in range(NCH):
                        c = slice(hh * CW, (hh + 1) * CW)
                        rs = tq.tile([128, CW], F32, tag="rs")
                        nc.vector.tensor_scalar(rs[:P, :], mem[:, c], TH1, TH1, op0=GT, op1=MUL)
                        nc.vector.tensor_scalar(mem[:, c], mem[:, c], BETA, None, op0=MUL)
                        nc.vector.tensor_tensor(mem[:, c], mem[:, c], cur[:, c], op=ADD)
                        nc.vector.tensor_tensor(mem[:, c], mem[:, c], rs[:P, :], op=SUB)
                        nc.vector.tensor_scalar(spk[:, c], mem[:, c], TH1, None, op0=GT)

                # ---- conv2 + pool + BN2 + LIF2 + FC
                pfc = pp.tile([10, B], F32, tag="pfc")
                for xp in range(5):
                    px03 = tr.tile([128, B], F32, tag="cpx0")
                    px47 = tr.tile([128, B], F32, tag="cpx1")
                    px89 = tr.tile([128, B], F32, tag="cpx2")
                    for xo in range(2):
                        x = 2 * xp + xo
                        p03 = pp.tile([128, B], F32, tag="p03")
                        p47 = pp.tile([128, B], F32, tag="p47")
                        p89 = pp.tile([128, B], F32, tag="p89")
                        for dx in range(3):
                            Xd = slice((x + dx) * B, (x + dx + 1) * B)
                            nc.tensor.matmul(p03[:], w03[dx][:], spk1a[0:96, Xd],
                                             start=(dx == 0), stop=(dx == 2))
                            nc.tensor.matmul(p47[:], w47[dx][64:128, :], spk1a[64:128, Xd],
                                             start=(dx == 0), stop=False)
                            nc.tensor.matmul(p47[:], w47[dx][0:32, :], spk1b[0:32, Xd],
                                             start=False, stop=(dx == 2))
                            nc.tensor.matmul(p89[0:64, :], w89[dx][:], spk1b[0:64, Xd],
                                             start=(dx == 0), stop=(dx == 2))
                        if xo == 0:
                            nc.scalar.copy(px03[:], p03[:])
                            nc.scalar.copy(px47[:], p47[:])
                            nc.scalar.copy(px89[0:64, :], p89[0:64, :])
                        else:
                            nc.vector.tensor_tensor(px03[:], px03[:], p03[:], op=MAX)
                            nc.vector.tensor_tensor(px47[:], px47[:], p47[:], op=MAX)
                            nc.vector.tensor_tensor(px89[0:64, :], px89[0:64, :], p89[0:64, :], op=MAX)
                    xs = slice(xp * B, (xp + 1) * B)
                    first = (xp == 0)
                    plt = tr.tile([128, B], F32, tag="pl")
                    rs2t = tr.tile([128, B], F32, tag="rs2")
                    spk2t = tr.tile([128, B], BF16, tag="spk2")
                    od2 = tr.tile([64, B], F32, tag="od2")
                    for gi, (pxt, m2g, wfct, sl, gp) in enumerate((
                            (px03, m2ab[0:64, xs], wfc0123, slice(0, 64), 64),
                            (px47, m2ab[64:128, xs], wfc0123, slice(64, 128), 64),
                            (px89, m2c[0:32, xs], wfc4, slice(0, 32), 32))):
                        nc.vector.tensor_copy(od2[0:gp, :], pxt[gp:2 * gp, :])
                        nc.vector.tensor_tensor(plt[sl, :], pxt[0:gp, :], od2[0:gp, :], op=MAX)
                        # BN2: (k - m) * s + b
                        nc.vector.tensor_scalar(plt[sl, :], plt[sl, :],
                                                aux32[sl, 3:4], aux32[sl, 4:5],
                                                op0=SUB, op1=MUL)
                        nc.vector.tensor_scalar(plt[sl, :], plt[sl, :],
                                                aux32[sl, 5:6], None, op0=ADD)
                        # LIF2
                        nc.vector.tensor_scalar(rs2t[sl, :], m2g, TH2, TH2, op0=GT, op1=MUL)
                        nc.vector.tensor_scalar(m2g, m2g, BETA, None, op0=MUL)
                        nc.vector.tensor_tensor(m2g, m2g, plt[sl, :], op=ADD)
                        nc.vector.tensor_tensor(m2g, m2g, rs2t[sl, :], op=SUB)
                        nc.vector.tensor_scalar(spk2t[sl, :], m2g, TH2, None, op0=GT)
                        nc.tensor.matmul(pfc[:], wfct[sl, 10 * xp:10 * xp + 10], spk2t[sl, :],
                                         start=(first and gi == 0),
                                         stop=(xp == 4 and gi == 2))

                # ---- LIF3 + record
                c3 = tr.tile([10, B], F32, tag="c3")
                nc.vector.tensor_scalar(c3[:], pfc[:], aux32[0:10, 6:7], None, op0=ADD)
                rs3 = tr.tile([10, B], F32, tag="rs3")
                nc.vector.tensor_scalar(rs3[:], mem3[:], TH3, TH3, op0=GT, op1=MUL)
                nc.vector.tensor_scalar(mem3[:], mem3[:], BETA, None, op0=MUL)
                nc.vector.tensor_tensor(mem3[:], mem3[:], c3[:], op=ADD)
                nc.vector.tensor_tensor(mem3[:], mem3[:], rs3[:], op=SUB)
                spb = op.tile([10, B], F32, tag="spb")
                if t < 8:
                    nc.vector.tensor_scalar(spb[:], mem3[:], TH3, float(1 << t), op0=GT, op1=MUL)
                    nc.vector.tensor_tensor(acc07[:], acc07[:], spb[:], op=ADD)
                else:
                    nc.vector.tensor_scalar(spb[:], mem3[:], TH3, float(1 << (t - 8)), op0=GT, op1=MUL)
                    nc.vector.tensor_tensor(acc89[:], acc89[:], spb[:], op=ADD)
                mo = op.tile([10, B], BF16, tag="mo")
                nc.vector.tensor_copy(mo[:], mem3[:])
                nc.sync.dma_start(out_d[10 * t:10 * (t + 1)], mo[:])

            b07 = op.tile([10, B], BF16, tag="b07")
            nc.vector.tensor_copy(b07[:], acc07[:])
            nc.sync.dma_start(out_d[100:110], b07[:])
            b89 = op.tile([10, B], BF16, tag="b89")
            nc.vector.tensor_copy(b89[:], acc89[:])
            nc.sync.dma_start(out_d[110:120], b89[:])

    nc.compile()
    return nc


def kernel(inpt, w1, w2, w_fc, b_fc, bn1_g, bn1_b, bn1_m, bn1_v,
           bn2_g, bn2_b, bn2_m, bn2_v):
    inpt = np.asarray(inpt, np.float32)
    w1 = np.asarray(w1, np.float32); w2 = np.asarray(w2, np.float32)
    w_fc = np.asarray(w_fc, np.float32); b_fc = np.asarray(b_fc, np.float32)
    bn1_g = np.asarray(bn1_g, np.float32); bn1_b = np.asarray(bn1_b, np.float32)
    bn1_m = np.asarray(bn1_m, np.float32); bn1_v = np.asarray(bn1_v, np.float32)
    bn2_g = np.asarray(bn2_g, np.float32); bn2_b = np.asarray(bn2_b, np.float32)
    bn2_m = np.asarray(bn2_m, np.float32); bn2_v = np.asarray(bn2_v, np.float32)

    bw1 = np.sign(w1).astype(np.float32)
    bw2 = np.sign(w2).astype(np.float32)
    bwfc = np.sign(w_fc).astype(np.float32)
    s1 = (bn1_g * (np.float32(1.0) / np.sqrt(bn1_v + EPS, dtype=np.float32))).astype(np.float32)
    s2 = (bn2_g * (np.float32(1.0) / np.sqrt(bn2_v + EPS, dtype=np.float32))).astype(np.float32)

    # compact weight sources; Toeplitz blocks are assembled on device via DMA
    bw1c = np.ascontiguousarray(bw1[:, 0].transpose(2, 1, 0))          # [dx, dyy, co]
    bw2c = np.ascontiguousarray(bw2.transpose(3, 2, 1, 0)).reshape(3, 48, 32)

    aux32 = np.zeros((128, 7), np.float32)
    aux32[:, 0] = np.tile(bn1_m, 8); aux32[:, 1] = np.tile(s1, 8); aux32[:, 2] = np.tile(bn1_b, 8)
    aux32[:, 3] = np.tile(bn2_m, 4); aux32[:, 4] = np.tile(s2, 4); aux32[:, 5] = np.tile(bn2_b, 4)
    aux32[0:10, 6] = b_fc

    wfc_r = bwfc.reshape(10, 32, 5, 5)
    def fcblock(yps):
        W = np.zeros((len(yps) * 32, 50), np.float32)
        for i, yp in enumerate(yps):
            W[i * 32:(i + 1) * 32] = wfc_r[:, :, yp, :].transpose(1, 2, 0).reshape(32, 50)
        return W.astype(BF)
    auxbf = np.zeros((128, 196), BF)
    for dx in range(3):
        auxbf[0:48, dx * 32:(dx + 1) * 32] = bw2c[dx]
    auxbf[:, 96:146] = np.vstack([fcblock([0, 1]), fcblock([2, 3])])
    auxbf[0:32, 146:196] = fcblock([4])

    if 'nc' not in _cache:
        _cache['nc'] = _build_program()
    nc = _cache['nc']

    XT = np.ascontiguousarray(inpt[:, 0, 0:26, 0:26].transpose(1, 2, 0))  # [26,26,Bfull]
    in_maps = []
    for c in range(NCORES):
        xc = np.ascontiguousarray(XT[:, :, c * B:(c + 1) * B]).reshape(26, 26 * B)
        in_maps.append({
            "x26": xc, "bw1c": bw1c, "aux32": aux32, "auxbf": auxbf,
        })

    import time as _time
    _t0 = _time.perf_counter()
    res = run_bass_kernel_spmd(nc, in_maps, list(range(NCORES)))
    _t1 = _time.perf_counter()
    global LAST_RES, LAST_NS
    LAST_RES = res
    LAST_NS = (_t1 - _t0) * 1e9
    arr = np.stack([np.asarray(r["out"], np.float32) for r in res.results])  # [8,120,512]
    mem = np.ascontiguousarray(
        arr[:, 0:100].reshape(NCORES, NSTEP, 10, B).transpose(1, 0, 3, 2).reshape(NSTEP, NCORES * B, 10))
    bits = np.concatenate([arr[:, 100:110], arr[:, 110:120]], axis=0).astype(np.int32)  # [16,10,512]
    spk = np.empty((NSTEP, NCORES * B, 10), np.float32)
    for t in range(NSTEP):
        src = bits[0:NCORES] if t < 8 else bits[NCORES:]
        sh = t if t < 8 else t - 8
        spk[t] = ((src >> sh) & 1).transpose(0, 2, 1).reshape(NCORES * B, 10).astype(np.float32)
    return spk, mem


if __name__ == "__main__":
    pass


# revision 42
# speedup vs baseline: 1.5508x; 1.5508x over previous
import numpy as np
import sys
sys.path.insert(0, '/opt/trn_rl_repo')
import jax
try:
    jax.config.update("jax_compilation_cache_dir", "/tmp/jaxcache")
    jax.config.update("jax_persistent_cache_min_compile_time_secs", 0.0)
    jax.config.update("jax_persistent_cache_min_entry_size_bytes", 0)
except Exception:
    pass
import concourse.bacc as bacc
import concourse.mybir as mybir
from concourse.tile import TileContext
from concourse.bass import ds
from concourse.bass_utils import run_bass_kernel_spmd
import concourse.tile_utils as tile_utils
tile_utils.max_sbuf_usage = 207 * 1024

import ml_dtypes
BF = ml_dtypes.bfloat16

TH1 = 2.3599835635698114
TH2 = 7.985043705972782
TH3 = 3.849629060468402
BETA = 0.44154740154430405
EPS = 1e-5
NSTEP = 10
NCORES = 8
B = 512            # batch per core
F32 = mybir.dt.float32
BF16 = mybir.dt.bfloat16

# Even conv-rows first inside each 8-row block so 2x2 pool-y is a single
# quadrant-aligned max(p[0:64], p[64:128]).
PERM8 = (0, 2, 4, 6, 1, 3, 5, 7)

_cache = {}
LAST_RES = None
LAST_NS = -1


def _build_program():
    nc = bacc.Bacc("TRN2", target_bir_lowering=False, debug=False, num_devices=NCORES)

    x26_d = nc.dram_tensor("x26", [26, 26 * B], F32, kind="ExternalInput")
    bw1c_d = nc.dram_tensor("bw1c", [3, 3, 16], F32, kind="ExternalInput")
    # aux32 cols: 0:3 bn1 (m,s,b), 3:6 bn2, 6 b_fc (rows 0:10)
    aux32_d = nc.dram_tensor("aux32", [128, 7], F32, kind="ExternalInput")
    # auxbf cols: 0:96 bw2c (3 x [48,32], rows 0:48), 96:146 wfc0123, 146:196 wfc4 (rows 0:32)
    auxbf_d = nc.dram_tensor("auxbf", [128, 196], BF16, kind="ExternalInput")
    # rows 0..99: mem3 per step; 100..109: spike bits t=0..7; 110..119: t=8,9
    out_d = nc.dram_tensor("out", [120, B], BF16, kind="ExternalOutput")

    GT, MUL, ADD, SUB, MAX = (mybir.AluOpType.is_gt, mybir.AluOpType.mult,
                              mybir.AluOpType.add, mybir.AluOpType.subtract,
                              mybir.AluOpType.max)
    F = 12 * B         # LIF1 free width: X' 0..11 (X'=12 never read by conv2)

    with TileContext(nc) as tc:
        with (
            tc.tile_pool(name="state", bufs=1) as st,
            tc.tile_pool(name="xt", bufs=1) as xpool,
            tc.tile_pool(name="tr", bufs=1) as tr,
            tc.tile_pool(name="tq", bufs=1) as tq,
            tc.tile_pool(name="outp", bufs=1) as op,
            tc.tile_pool(name="ps", bufs=2, space="PSUM") as pp,
        ):
            cur1a = st.tile([128, F], F32)      # Y' 0..7 x16ch
            cur1b = st.tile([64, F], F32)       # Y' 8..11
            mem1a = st.tile([128, F], F32)
            mem1b = st.tile([64, F], F32)
            spk1a = st.tile([128, F], BF16)
            spk1b = st.tile([64, F], BF16)
            m2ab = st.tile([128, 5 * B], F32)   # Y2 0,1 | 2,3
            m2c = st.tile([32, 5 * B], F32)     # Y2 4
            mem3 = st.tile([10, B], F32)

            aux32 = st.tile([128, 7], F32)
            nc.sync.dma_start(aux32[:], aux32_d[:])

            # conv1 stationaries built on device from the 144-value bw1c
            wc1t = st.tile([26, 9 * 128], F32)
            nc.vector.memset(wc1t[:], 0.0)
            for bi, y0 in enumerate((0, 8, 16)):
                for dx in range(3):
                    cb = (bi * 3 + dx) * 128
                    for p_, j in enumerate(PERM8):
                        nc.sync.dma_start(
                            wc1t[y0 + j:y0 + j + 3, cb + p_ * 16:cb + (p_ + 1) * 16],
                            bw1c_d[dx])

            # conv2 Toeplitz blocks built on device from auxbf cols 0:96
            w03, w47, w89 = [], [], []
            for dx in range(3):
                ws = slice(dx * 32, (dx + 1) * 32)
                t1 = st.tile([96, 128], BF16, tag=f"w03_{dx}")
                nc.vector.memset(t1[:], 0.0)
                for pos, yr in enumerate((0, 2, 1, 3)):
                    nc.sync.dma_start(t1[yr * 16:yr * 16 + 48, pos * 32:(pos + 1) * 32],
                                      auxbf_d[0:48, ws])
                w03.append(t1)
                # rows for Y 4..7 live at partitions 64..127, Y 8..9 at 0..31
                t2 = st.tile([128, 128], BF16, tag=f"w47_{dx}")
                nc.vector.memset(t2[:], 0.0)
                for pos, yr in enumerate((0, 2, 1, 3)):
                    r0 = yr * 16          # block row where this 48-row strip starts
                    cs = slice(pos * 32, (pos + 1) * 32)
                    lo = min(48, 64 - r0)  # rows landing in block rows <64 -> +64
                    if lo > 0:
                        nc.sync.dma_start(t2[64 + r0:64 + r0 + lo, cs], auxbf_d[0:lo, ws])
                    if lo < 48:
                        nc.sync.dma_start(t2[r0 + lo - 64:r0 + 48 - 64, cs], auxbf_d[lo:48, ws])
                w47.append(t2)
                t3 = st.tile([64, 64], BF16, tag=f"w89_{dx}")
                nc.vector.memset(t3[:], 0.0)
                for pos, yr in enumerate((0, 1)):
                    nc.sync.dma_start(t3[yr * 16:yr * 16 + 48, pos * 32:(pos + 1) * 32],
                                      auxbf_d[0:48, ws])
                w89.append(t3)
            wfc0123 = st.tile([128, 50], BF16)
            nc.sync.dma_start(wfc0123[:], auxbf_d[:, 96:146])
            wfc4 = st.tile([32, 50], BF16)
            nc.sync.dma_start(wfc4[:], auxbf_d[0:32, 146:196])
            accA = st.tile([10, B], F32)
            nc.vector.memset(accA[:], 0.0)
            bitval = st.tile([10, 1], F32)
            nc.vector.memset(bitval[:], 1.0)

            nc.vector.memset(mem1a[:], 0.0)
            nc.vector.memset(mem1b[:], 0.0)
            nc.vector.memset(m2ab[:], 0.0)
            nc.vector.memset(m2c[:], 0.0)
            nc.vector.memset(mem3[:], 0.0)

            # ---- conv1 + 2x2 pool, on-device; three x-thirds to bound SBUF
            for h in range(3):
                xoff = 8 * h                    # x_in of tile column 0
                x26t = xpool.tile([26, 10 * B], F32, tag="x26")
                nc.sync.dma_start(x26t[:], x26_d[:, xoff * B:(xoff + 10) * B])
                for xp in range(4 * h, 4 * (h + 1)):
                    px0 = tr.tile([128, B], F32, tag="cpx0")
                    px1 = tr.tile([128, B], F32, tag="cpx1")
                    px2 = tr.tile([128, B], F32, tag="cpx2")
                    px = (px0, px1, px2)
                    for xo in range(2):
                        x = 2 * xp + xo
                        for bi in range(3):
                            p = pp.tile([128, B], F32, tag=("p03", "p47", "p89")[bi])
                            for dx in range(3):
                                ci = (x + dx - xoff) * B
                                nc.tensor.matmul(
                                    p[:],
                                    wc1t[:, (bi * 3 + dx) * 128:(bi * 3 + dx + 1) * 128],
                                    x26t[:, ci:ci + B],
                                    start=(dx == 0), stop=(dx == 2))
                            if xo == 0:
                                nc.scalar.copy(px[bi][:], p[:])
                            else:
                                nc.vector.tensor_tensor(px[bi][:], px[bi][:], p[:], op=MAX)
                    xs = slice(xp * B, (xp + 1) * B)
                    odc = tr.tile([64, B], F32, tag="odc")
                    for bi, dest in ((0, cur1a[0:64, xs]),
                                     (1, cur1a[64:128, xs]),
                                     (2, cur1b[0:64, xs])):
                        nc.vector.tensor_copy(odc[:], px[bi][64:128, :])
                        nc.vector.tensor_tensor(dest, px[bi][0:64, :], odc[:], op=MAX)

            # BN1 (pool-before-BN is exact: s1 >= 0)
            nc.vector.tensor_scalar(cur1a[:], cur1a[:], aux32[:, 0:1], aux32[:, 1:2], op0=SUB, op1=MUL)
            nc.vector.tensor_scalar(cur1a[:], cur1a[:], aux32[:, 2:3], None, op0=ADD)
            nc.vector.tensor_scalar(cur1b[:], cur1b[:], aux32[0:64, 0:1], aux32[0:64, 1:2], op0=SUB, op1=MUL)
            nc.vector.tensor_scalar(cur1b[:], cur1b[:], aux32[0:64, 2:3], None, op0=ADD)

            NCH = 4
            CW = F // NCH
            with tc.For_i(0, 10 * NSTEP, 10) as tv:
                # ---- LIF1
                for (mem, cur, spk, P) in ((mem1a, cur1a, spk1a, 128),
                                           (mem1b, cur1b, spk1b, 64)):
                    for hh in range(NCH):
                        c = slice(hh * CW, (hh + 1) * CW)
                        rs = tq.tile([128, CW], F32, tag="rs")
                        nc.vector.tensor_scalar(rs[:P, :], mem[:, c], TH1, TH1, op0=GT, op1=MUL)
                        nc.vector.tensor_scalar(mem[:, c], mem[:, c], BETA, None, op0=MUL)
                        nc.vector.tensor_tensor(mem[:, c], mem[:, c], cur[:, c], op=ADD)
                        nc.vector.tensor_tensor(mem[:, c], mem[:, c], rs[:P, :], op=SUB)
                        nc.vector.tensor_scalar(spk[:, c], mem[:, c], TH1, None, op0=GT)

                # ---- conv2 + pool + BN2 + LIF2 + FC
                pfc = pp.tile([10, B], F32, tag="pfc")
                for xp in range(5):
                    px03 = tr.tile([128, B], F32, tag="cpx0")
                    px47 = tr.tile([128, B], F32, tag="cpx1")
                    px89 = tr.tile([128, B], F32, tag="cpx2")
                    for xo in range(2):
                        x = 2 * xp + xo
                        p03 = pp.tile([128, B], F32, tag="p03")
                        p47 = pp.tile([128, B], F32, tag="p47")
                        p89 = pp.tile([128, B], F32, tag="p89")
                        for dx in range(3):
                            Xd = slice((x + dx) * B, (x + dx + 1) * B)
                            nc.tensor.matmul(p03[:], w03[dx][:], spk1a[0:96, Xd],
                                             start=(dx == 0), stop=(dx == 2))
                            nc.tensor.matmul(p47[:], w47[dx][64:128, :], spk1a[64:128, Xd],
                                             start=(dx == 0), stop=False)
                            nc.tensor.matmul(p47[:], w47[dx][0:32, :], spk1b[0:32, Xd],
                                             start=False, stop=(dx == 2))
                            nc.tensor.matmul(p89[0:64, :], w89[dx][:], spk1b[0:64, Xd],
                                             start=(dx == 0), stop=(dx == 2))
                        if xo == 0:
                            nc.scalar.copy(px03[:], p03[:])
                            nc.scalar.copy(px47[:], p47[:])
                            nc.scalar.copy(px89[0:64, :], p89[0:64, :])
                        else:
                            nc.vector.tensor_tensor(px03[:], px03[:], p03[:], op=MAX)
                            nc.vector.tensor_tensor(px47[:], px47[:], p47[:], op=MAX)
                            nc.vector.tensor_tensor(px89[0:64, :], px89[0:64, :], p89[0:64, :], op=MAX)
                    xsg = slice(xp * B, (xp + 1) * B)
                    first = (xp == 0)
                    plt = tr.tile([128, B], F32, tag="pl")
                    rs2t = tr.tile([128, B], F32, tag="rs2")
                    spk2t = tr.tile([128, B], BF16, tag="spk2")
                    od2 = tr.tile([64, B], F32, tag="od2")
                    for gi, (pxt, m2g, wfct, sl, gp) in enumerate((
                            (px03, m2ab[0:64, xsg], wfc0123, slice(0, 64), 64),
                            (px47, m2ab[64:128, xsg], wfc0123, slice(64, 128), 64),
                            (px89, m2c[0:32, xsg], wfc4, slice(0, 32), 32))):
                        nc.vector.tensor_copy(od2[0:gp, :], pxt[gp:2 * gp, :])
                        nc.vector.tensor_tensor(plt[sl, :], pxt[0:gp, :], od2[0:gp, :], op=MAX)
                        # BN2: (k - m) * s + b
                        nc.vector.tensor_scalar(plt[sl, :], plt[sl, :],
                                                aux32[sl, 3:4], aux32[sl, 4:5],
                                                op0=SUB, op1=MUL)
                        nc.vector.tensor_scalar(plt[sl, :], plt[sl, :],
                                                aux32[sl, 5:6], None, op0=ADD)
                        # LIF2
                        nc.vector.tensor_scalar(rs2t[sl, :], m2g, TH2, TH2, op0=GT, op1=MUL)
                        nc.vector.tensor_scalar(m2g, m2g, BETA, None, op0=MUL)
                        nc.vector.tensor_tensor(m2g, m2g, plt[sl, :], op=ADD)
                        nc.vector.tensor_tensor(m2g, m2g, rs2t[sl, :], op=SUB)
                        nc.vector.tensor_scalar(spk2t[sl, :], m2g, TH2, None, op0=GT)
                        nc.tensor.matmul(pfc[:], wfct[sl, 10 * xp:10 * xp + 10], spk2t[sl, :],
                                         start=(first and gi == 0),
                                         stop=(xp == 4 and gi == 2))

                # ---- LIF3 + record
                c3 = tr.tile([10, B], F32, tag="c3")
                nc.vector.tensor_scalar(c3[:], pfc[:], aux32[0:10, 6:7], None, op0=ADD)
                rs3 = tr.tile([10, B], F32, tag="rs3")
                nc.vector.tensor_scalar(rs3[:], mem3[:], TH3, TH3, op0=GT, op1=MUL)
                nc.vector.tensor_scalar(mem3[:], mem3[:], BETA, None, op0=MUL)
                nc.vector.tensor_tensor(mem3[:], mem3[:], c3[:], op=ADD)
                nc.vector.tensor_tensor(mem3[:], mem3[:], rs3[:], op=SUB)
                # spike bits: accA += (mem3 > TH3) * 2^t, exact in f32 (sum <= 1023)
                spb = op.tile([10, B], F32, tag="spb")
                nc.vector.tensor_scalar(spb[:], mem3[:], TH3, bitval[:, 0:1], op0=GT, op1=MUL)
                nc.vector.tensor_tensor(accA[:], accA[:], spb[:], op=ADD)
                nc.vector.tensor_scalar(bitval[:], bitval[:], 2.0, None, op0=MUL)
                mo = op.tile([10, B], BF16, tag="mo")
                nc.vector.tensor_copy(mo[:], mem3[:])
                nc.sync.dma_start(out_d[ds(tv, 10)], mo[:])

            # split accA (<=1023) into bits 0..7 (<=255, bf16-exact) and bits 8,9
            t9 = op.tile([10, B], F32, tag="t9")
            nc.vector.tensor_scalar(t9[:], accA[:], 511.5, 512.0, op0=GT, op1=MUL)
            nc.vector.tensor_tensor(accA[:], accA[:], t9[:], op=SUB)
            t8 = op.tile([10, B], F32, tag="t8")
            nc.vector.tensor_scalar(t8[:], accA[:], 255.5, 256.0, op0=GT, op1=MUL)
            nc.vector.tensor_tensor(accA[:], accA[:], t8[:], op=SUB)
            b07 = op.tile([10, B], BF16, tag="b07")
            nc.vector.tensor_copy(b07[:], accA[:])
            nc.sync.dma_start(out_d[100:110], b07[:])
            nc.vector.tensor_tensor(t8[:], t8[:], t9[:], op=ADD)
            b89 = op.tile([10, B], BF16, tag="b89")
            nc.vector.tensor_scalar(b89[:], t8[:], 1.0 / 256.0, None, op0=MUL)
            nc.sync.dma_start(out_d[110:120], b89[:])

    nc.compile()
    return nc


def kernel(inpt, w1, w2, w_fc, b_fc, bn1_g, bn1_b, bn1_m, bn1_v,
           bn2_g, bn2_b, bn2_m, bn2_v):
    inpt = np.asarray(inpt, np.float32)
    w1 = np.asarray(w1, np.float32); w2 = np.asarray(w2, np.float32)
    w_fc = np.asarray(w_fc, np.float32); b_fc = np.asarray(b_fc, np.float32)
    bn1_g = np.asarray(bn1_g, np.float32); bn1_b = np.asarray(bn1_b, np.float32)
    bn1_m = np.asarray(bn1_m, np.float32); bn1_v = np.asarray(bn1_v, np.float32)
    bn2_g = np.asarray(bn2_g, np.float32); bn2_b = np.asarray(bn2_b, np.float32)
    bn2_m = np.asarray(bn2_m, np.float32); bn2_v = np.asarray(bn2_v, np.float32)

    bw1 = np.sign(w1).astype(np.float32)
    bw2 = np.sign(w2).astype(np.float32)
    bwfc = np.sign(w_fc).astype(np.float32)
    s1 = (bn1_g * (np.float32(1.0) / np.sqrt(bn1_v + EPS, dtype=np.float32))).astype(np.float32)
    s2 = (bn2_g * (np.float32(1.0) / np.sqrt(bn2_v + EPS, dtype=np.float32))).astype(np.float32)

    # compact weight sources; Toeplitz blocks are assembled on device via DMA
    bw1c = np.ascontiguousarray(bw1[:, 0].transpose(2, 1, 0))          # [dx, dyy, co]
    bw2c = np.ascontiguousarray(bw2.transpose(3, 2, 1, 0)).reshape(3, 48, 32)

    aux32 = np.zeros((128, 7), np.float32)
    aux32[:, 0] = np.tile(bn1_m, 8); aux32[:, 1] = np.tile(s1, 8); aux32[:, 2] = np.tile(bn1_b, 8)
    aux32[:, 3] = np.tile(bn2_m, 4); aux32[:, 4] = np.tile(s2, 4); aux32[:, 5] = np.tile(bn2_b, 4)
    aux32[0:10, 6] = b_fc

    wfc_r = bwfc.reshape(10, 32, 5, 5)
    def fcblock(yps):
        W = np.zeros((len(yps) * 32, 50), np.float32)
        for i, yp in enumerate(yps):
            W[i * 32:(i + 1) * 32] = wfc_r[:, :, yp, :].transpose(1, 2, 0).reshape(32, 50)
        return W.astype(BF)
    auxbf = np.zeros((128, 196), BF)
    for dx in range(3):
        auxbf[0:48, dx * 32:(dx + 1) * 32] = bw2c[dx]
    auxbf[:, 96:146] = np.vstack([fcblock([0, 1]), fcblock([2, 3])])
    auxbf[0:32, 146:196] = fcblock([4])

    if 'nc' not in _cache:
        _cache['nc'] = _build_program()
    nc = _cache['nc']

    XT = np.ascontiguousarray(inpt[:, 0, 0:26, 0:26].transpose(1, 2, 0))  # [26,26,Bfull]
    in_maps = []
    for c in range(NCORES):
        xc = np.ascontiguousarray(XT[:, :, c * B:(c + 1) * B]).reshape(26, 26 * B)
        in_maps.append({
            "x26": xc, "bw1c": bw1c, "aux32": aux32, "auxbf": auxbf,
        })

    import time as _time
    _t0 = _time.perf_counter()
    res = run_bass_kernel_spmd(nc, in_maps, list(range(NCORES)))
    _t1 = _time.perf_counter()
    global LAST_RES, LAST_NS
    LAST_RES = res
    LAST_NS = (_t1 - _t0) * 1e9
    arr = np.stack([np.asarray(r["out"], np.float32) for r in res.results])  # [8,120,512]
    mem = np.ascontiguousarray(
        arr[:, 0:100].reshape(NCORES, NSTEP, 10, B).transpose(1, 0, 3, 2).reshape(NSTEP, NCORES * B, 10))
    bits = np.concatenate([arr[:, 100:110], arr[:, 110:120]], axis=0).astype(np.int32)  # [16,10,512]
    spk = np.empty((NSTEP, NCORES * B, 10), np.float32)
    for t in range(NSTEP):
        src = bits[0:NCORES] if t < 8 else bits[NCORES:]
        sh = t if t < 8 else t - 8
        spk[t] = ((src >> sh) & 1).transpose(0, 2, 1).reshape(NCORES * B, 10).astype(np.float32)
    return spk, mem


if __name__ == "__main__":
    pass
